# revision 8
# baseline (speedup 1.0000x reference)
"""GCN (2x GCNConv + linear head) on 8 TRN2 NeuronCores — bf16 pipeline.

Strategy (graph-parallel by target node):
- Nodes sharded across 8 cores (6250 real + pad = 6400 rows/core,
  table_row = core*6400 + local).
- Layer tables H = dis_src * (x @ W) live in DRAM as bf16 rows (256B),
  built as local shards then AllGathered (both layers).
- Edges grouped per (core, 256-target window, lo/hi source range); the
  lo/hi split works around dma_gather's int16 index range.  Each
  128-edge block becomes one matmul: PSUM[feat, tgt] +=
  gathered[edge, feat].T @ onehot[edge, tgt].  The 0/1 one-hot blocks
  are precomputed host-side in fp8 (1.0 is exact) and streamed from
  DRAM — no DVE work per block.  Per-window drain applies the
  target-deg scale (tensor_tensor with a broadcast rsqrt(deg) tile),
  then ReLU+bias on the Scalar engine.  Self-loops are ordinary edges.
- Window block counts vary per window (max over cores keeps the SPMD
  program identical); gather calls pack GCH blocks flat across windows.
- Host prep is integer-only; rsqrt(deg) is computed on device.
"""

import numpy as np
from ml_dtypes import bfloat16, float8_e4m3

N_REAL = 50000
E_REAL = 800000
D = 128
NCORES = 8
GCH = 8  # blocks per dma_gather call (1024 idxs: swdge firmware limit)
PHASES = {"A", "G1", "S", "G2", "H"}

_CFG_FULL = dict(n=N_REAL, nsh=6250, win=256, split=32768)


def _derive(cfg):
    nsh = cfg["nsh"]
    nloc = ((nsh + 127) // 128) * 128
    win = cfg["win"]
    nloc = ((nloc + win - 1) // win) * win
    npad = NCORES * nloc
    return nloc, npad, nloc // win, npad // 128


def _calls(nb):
    """Pack nb blocks into dma_gather calls of <= GCH blocks."""
    out = []
    b0 = 0
    while b0 < nb:
        out.append((b0, min(GCH, nb - b0)))
        b0 += GCH
    return out


def prep(edge_index, cfg=_CFG_FULL):
    """Host-side (integer-only) graph preprocessing -> per-core arrays."""
    n, nsh, win, split = cfg["n"], cfg["nsh"], cfg["win"], cfg["split"]
    nloc, npad, nwin, _ = _derive(cfg)
    row = np.asarray(edge_index[0]).astype(np.int64)
    col = np.asarray(edge_index[1]).astype(np.int64)

    deg = np.bincount(col, minlength=n).astype(np.float32) + 1.0  # + self loop
    deg_t = np.ones(npad, np.float32)
    rr = np.arange(n, dtype=np.int64)
    t_of_r = (rr // nsh) * nloc + (rr % nsh)
    deg_t[t_of_r] = deg

    trow = (row // nsh) * nloc + (row % nsh)
    tcol = (col // nsh) * nloc + (col % nsh)
    core_of = col // nsh

    # per-core/per-class edge lists sorted by window
    percls = []  # [core][cls] = (src_rows, col_in_window, window_bounds)
    for c in range(NCORES):
        m = core_of == c
        er = np.concatenate([trow[m], c * nloc + np.arange(nsh, dtype=np.int64)])
        ecl = np.concatenate([tcol[m] - c * nloc, np.arange(nsh, dtype=np.int64)])
        w = ecl // win
        is_hi = er >= split
        cls_list = []
        for cls in (0, 1):
            mm = is_hi == bool(cls)
            erc, eclc, wc = er[mm], ecl[mm], w[mm]
            order = np.argsort(wc, kind="stable")
            erc, eclc, wc = erc[order], eclc[order], wc[order]
            bounds = np.searchsorted(wc, np.arange(nwin + 1))
            cls_list.append((erc, eclc, bounds))
        percls.append(cls_list)

    # per-(class, window) block count = max over cores (keeps SPMD structure)
    nbw = np.zeros((2, nwin), np.int64)
    for cls in (0, 1):
        for w in range(nwin):
            mx = max(percls[c][cls][2][w + 1] - percls[c][cls][2][w]
                     for c in range(NCORES))
            nbw[cls, w] = (mx + 127) // 128
    assert (nbw.sum(axis=0) > 0).all()

    starts = [np.concatenate([[0], np.cumsum(nbw[cls])]) for cls in (0, 1)]
    NB = [int(starts[cls][-1]) for cls in (0, 1)]

    one = np.uint8(0x38)  # 1.0 in float8_e4m3
    wins = np.arange(win, dtype=np.int64)

    cores = []
    for c in range(NCORES):
        arrs = {}
        for cls, sfx in ((0, "lo"), (1, "hi")):
            erc, eclc, bounds = percls[c][cls]
            nb = NB[cls]
            src = np.zeros((nb, 128), np.int64)
            cw = np.full((nb, 128), -1, np.int64)
            for w in range(nwin):
                a, b = bounds[w], bounds[w + 1]
                k = b - a
                base = starts[cls][w] * 128
                src.reshape(-1)[base:base + k] = erc[a:b] - (split if cls else 0)
                cw.reshape(-1)[base:base + k] = eclc[a:b] % win
            # gather idx layout per call: idx e -> part e%16 (replicated x8),
            # col e//16
            gparts = []
            for b0, cs in _calls(nb):
                s = src[b0:b0 + cs].reshape(-1)
                g16 = s.reshape(-1, 16).T  # [16, cs*8]
                gparts.append(np.tile(g16, (8, 1)))
            arrs[f"gidx_{sfx}"] = np.concatenate(gparts, axis=1).astype(np.int16)
            # one-hot blocks in fp8: [128 part=edge, NB*win], block b at
            # cols [b*win,(b+1)*win)
            oh = (cw[:, :, None] == wins).astype(np.uint8) * one
            oh = oh.transpose(1, 0, 2).reshape(128, nb * win)
            arrs[f"oh_{sfx}"] = oh.view(float8_e4m3)
        # deg of this core's own table rows, [128, nloc/128] tiled
        arrs["degloc"] = deg_t[c * nloc:(c + 1) * nloc].reshape(-1, 128).T.copy()
        # deg of this core's targets broadcast across partitions
        arrs["degb"] = np.tile(deg_t[c * nloc:(c + 1) * nloc], (128, 1)).copy()
        cores.append(arrs)

    return cores, nbw, t_of_r


def build_nc(nbw, cfg=_CFG_FULL):
    import concourse.bacc as bacc
    import concourse.tile as tile
    import concourse.mybir as mybir
    from concourse.alu_op_type import AluOpType

    nloc, npad, nwin, ntile = _derive(cfg)
    win, split = cfg["win"], cfg["split"]
    f32 = mybir.dt.float32
    bf16 = mybir.dt.bfloat16
    f8 = mybir.dt.float8e4
    i16 = mybir.dt.int16
    AF = mybir.ActivationFunctionType
    nsh_t = nloc // 128

    starts = [np.concatenate([[0], np.cumsum(nbw[cls])]) for cls in (0, 1)]
    NB = [int(starts[cls][-1]) for cls in (0, 1)]
    calls = [_calls(NB[cls]) for cls in (0, 1)]
    call_of_block = []
    for cls in (0, 1):
        m = {}
        for j, (b0, cs) in enumerate(calls[cls]):
            for s in range(cs):
                m[b0 + s] = (j, s)
        call_of_block.append(m)

    nc = bacc.Bacc("TRN2", target_bir_lowering=False, debug=False,
                   num_devices=NCORES)
    inp = {}

    def I(name, shape, dt=f32):
        inp[name] = nc.dram_tensor(name, list(shape), dt, kind="ExternalInput").ap()
        return inp[name]

    xloc = I("xloc", [128, nloc], bf16)
    W1 = I("W1", [128, 128], bf16); W2 = I("W2", [128, 128], bf16)
    Wh = I("Wh", [128, 3], bf16)
    b1 = I("b1", [128, 1]); b2 = I("b2", [128, 1]); bh = I("bh", [128, 3])
    degloc = I("degloc", [128, nsh_t]); degb = I("degb", [128, nloc])
    g_lo = I("gidx_lo", [128, NB[0] * 8], i16)
    g_hi = I("gidx_hi", [128, NB[1] * 8], i16)
    oh_d = [I("oh_lo", [128, NB[0] * win], f8), I("oh_hi", [128, NB[1] * win], f8)]
    out = nc.dram_tensor("out", [128, nsh_t * 3], f32, kind="ExternalOutput").ap()

    bounce1 = nc.dram_tensor("bounce1", [nloc, 128], bf16, kind="Internal").ap()
    bounce2 = nc.dram_tensor("bounce2", [nloc, 128], bf16, kind="Internal").ap()
    T1 = nc.dram_tensor("T1", [npad, 128], bf16, kind="Internal",
                        addr_space="Shared").ap()
    T2 = nc.dram_tensor("T2", [npad, 128], bf16, kind="Internal",
                        addr_space="Shared").ap()

    with tile.TileContext(nc) as tc:
        with (
            tc.tile_pool(name="const", bufs=1) as pc,
            tc.tile_pool(name="ha", bufs=3) as pha,
            tc.tile_pool(name="glo", bufs=3) as pglo,
            tc.tile_pool(name="ghi", bufs=3) as pghi,
            tc.tile_pool(name="ohl", bufs=3) as pohl,
            tc.tile_pool(name="ohh", bufs=3) as pohh,
            tc.tile_pool(name="tmp", bufs=2) as ptmp,
            tc.tile_pool(name="act", bufs=1) as pact,
            tc.tile_pool(name="psA", bufs=2, space="PSUM") as psA,
            tc.tile_pool(name="psW", bufs=2, space="PSUM") as psW,
            tc.tile_pool(name="psH", bufs=2, space="PSUM") as psH,
        ):
            def load(ap, shape, tag, dt=f32):
                t = pc.tile(shape, dt, tag=tag)
                nc.sync.dma_start(t[:], ap[:])
                return t

            W1_sb = load(W1, [128, 128], "W1", bf16)
            W2_sb = load(W2, [128, 128], "W2", bf16)
            Wh_sb = load(Wh, [128, 3], "Wh", bf16)
            b1_sb = load(b1, [128, 1], "b1"); b2_sb = load(b2, [128, 1], "b2")
            bh_sb = load(bh, [128, 3], "bh")
            glo_sb = load(g_lo, [128, NB[0] * 8], "glosb", i16)
            ghi_sb = load(g_hi, [128, NB[1] * 8], "ghisb", i16)
            xloc_sb = load(xloc, [128, nloc], "xloc", bf16)

            def rsqrt_of(ap, cols, tag):
                dsb = load(ap, [128, cols], tag + "_d")
                nc.vector.reciprocal(dsb[:], dsb[:])
                nc.scalar.activation(dsb[:], dsb[:], AF.Sqrt)
                return dsb

            disloc_sb = rsqrt_of(degloc, nsh_t, "disl")
            disb_sb = rsqrt_of(degb, nloc, "disb")

            # persistent activations (feature-major)
            x2T = pact.tile([128, nloc], bf16, tag="x2T")
            x3T = pact.tile([128, nloc], bf16, tag="x3T")
            out_sb = pact.tile([128, nsh_t * 3], f32, tag="osb")

            # ---- local table shard: rows t*128..t*128+127 of this core ----
            def table_shard(src_sb, W_sb, bounce):
                for t in range(nsh_t):
                    ps = psA.tile([128, 128], f32, tag="psA")
                    nc.tensor.matmul(ps[:], src_sb[:, t * 128:(t + 1) * 128],
                                     W_sb[:], start=True, stop=True)
                    h = pha.tile([128, 128], bf16, tag="ha")
                    nc.vector.tensor_scalar(h[:], ps[:], disloc_sb[:, t:t + 1],
                                            None, AluOpType.mult)
                    nc.sync.dma_start(bounce[t * 128:(t + 1) * 128, :], h[:])

            def allgather(bounce, T):
                nc.gpsimd.collective_compute(
                    "AllGather", mybir.AluOpType.bypass,
                    replica_groups=[list(range(NCORES))],
                    ins=[bounce[:]], outs=[T[:]])

            if "A" in PHASES:
                table_shard(xloc_sb, W1_sb, bounce1)
                allgather(bounce1, T1)

            # ---- one GCN aggregation layer ----
            def agg_layer(T, xTnext, bias_sb):
                emitted = [{}, {}]  # cls -> call j -> (gather tile, onehot tile)

                def ensure_call(cls, j):
                    if j in emitted[cls]:
                        return emitted[cls][j]
                    b0, cs = calls[cls][j]
                    pl, pohx = (pglo, pohl) if cls == 0 else (pghi, pohh)
                    gsb = glo_sb if cls == 0 else ghi_sb
                    lim = (0, split) if cls == 0 else (split, npad)
                    gt = pl.tile([128, cs, 128], bf16, tag=f"g{cls}")
                    nc.gpsimd.dma_gather(
                        gt[:], T[lim[0]:lim[1], :], gsb[:, b0 * 8:(b0 + cs) * 8],
                        num_idxs=cs * 128, num_idxs_reg=cs * 128,
                        elem_size=128)
                    ot = pohx.tile([128, cs * win], f8, tag=f"o{cls}")
                    nc.sync.dma_start(ot[:], oh_d[cls][:, b0 * win:(b0 + cs) * win])
                    emitted[cls][j] = (gt, ot)
                    return gt, ot

                for w in range(nwin):
                    parts = []
                    for cls in (0, 1):
                        for b in range(int(starts[cls][w]), int(starts[cls][w + 1])):
                            j, slot = call_of_block[cls][b]
                            gt, ot = ensure_call(cls, j)
                            parts.append((gt, ot, slot))
                    acc = psW.tile([128, win], f32, tag="acc")
                    for k, (gt, ot, slot) in enumerate(parts):
                        nc.tensor.matmul(acc[:], gt[:, slot, :],
                                         ot[:, slot * win:(slot + 1) * win],
                                         start=(k == 0), stop=(k == len(parts) - 1))
                    tmp = ptmp.tile([128, win], bf16, tag="tmp")
                    nc.vector.tensor_tensor(tmp[:], acc[:],
                                            disb_sb[:, w * win:(w + 1) * win],
                                            AluOpType.mult)
                    nc.scalar.activation(xTnext[:, w * win:(w + 1) * win],
                                         tmp[:], AF.Relu, bias=bias_sb[:, 0:1])

            if "G1" in PHASES:
                agg_layer(T1, x2T, b1_sb)
            if "S" in PHASES:
                table_shard(x2T, W2_sb, bounce2)
                allgather(bounce2, T2)
            if "G2" in PHASES:
                agg_layer(T2, x3T, b2_sb)

            # ---- head ----
            for t in range(nsh_t if "H" in PHASES else 0):
                ps = psH.tile([128, 3], f32, tag="psH")
                nc.tensor.matmul(ps[:], x3T[:, t * 128:(t + 1) * 128], Wh_sb[:],
                                 start=True, stop=True)
                nc.vector.tensor_tensor(out_sb[:, t * 3:(t + 1) * 3], ps[:],
                                        bh_sb[:], AluOpType.add)
            nc.sync.dma_start(out[:], out_sb[:])

    nc.compile()
    return nc


def kernel(x, edge_index, W1, b1, W2, b2, Wh, bh, cfg=_CFG_FULL, _trace=False):
    from concourse.bass_utils import run_bass_kernel_spmd

    x = np.asarray(x, dtype=np.float32)
    W1 = np.asarray(W1, np.float32); b1 = np.asarray(b1, np.float32)
    W2 = np.asarray(W2, np.float32); b2 = np.asarray(b2, np.float32)
    Wh = np.asarray(Wh, np.float32); bh = np.asarray(bh, np.float32)
    n, nsh, win = cfg["n"], cfg["nsh"], cfg["win"]
    nloc, npad, nwin, ntile = _derive(cfg)

    cores, nbw, t_of_r = prep(edge_index, cfg)
    nc = build_nc(nbw, cfg)

    xTp = np.zeros((128, npad), np.float32)
    xTp[:, t_of_r] = x.T  # table-order, feature-major
    shared = dict(
        W1=W1.astype(bfloat16), W2=W2.astype(bfloat16),
        Wh=Wh.astype(bfloat16),
        b1=b1.reshape(128, 1), b2=b2.reshape(128, 1),
        bh=np.tile(bh.reshape(1, 3), (128, 1)).copy(),
    )
    in_maps = []
    for c in range(NCORES):
        m = dict(shared, **cores[c])
        m["xloc"] = xTp[:, c * nloc:(c + 1) * nloc].astype(bfloat16)
        in_maps.append(m)
    res = run_bass_kernel_spmd(nc, in_maps, core_ids=list(range(NCORES)),
                               trace=_trace)

    outs = []
    for c in range(NCORES):
        o = res.results[c]["out"].reshape(128, nloc // 128, 3)
        outs.append(o.transpose(1, 0, 2).reshape(nloc, 3)[:nsh])
    full = np.concatenate(outs, axis=0)[:n]
    if _trace:
        kernel.last_exec_ns = res.exec_time_ns
        kernel.last_trace = (res.instructions_and_trace or (None, None))[1]
    return full


# revision 11
# speedup vs baseline: 1.1655x; 1.1655x over previous
"""GCN (2x GCNConv + linear head) on 8 TRN2 NeuronCores — bf16 pipeline.

Strategy (graph-parallel by target node):
- Nodes sharded across 8 cores (6250 real + pad = 6400 rows/core,
  table_row = core*6400 + local).
- Layer tables H = dis_src * (x @ W) live in DRAM as bf16 rows (256B),
  built as local shards then AllGathered (both layers).
- Edges grouped per (core, 256-target window, lo/hi source range); the
  lo/hi split works around dma_gather's int16 index range.  Each
  128-edge block becomes one matmul: PSUM[feat, tgt] +=
  gathered[edge, feat].T @ onehot[edge, tgt].  The 0/1 one-hot blocks
  are precomputed host-side in fp8 (1.0 is exact) and streamed from
  DRAM — no DVE work per block.  Per-window drain applies the
  target-deg scale (tensor_tensor with a broadcast rsqrt(deg) tile),
  then ReLU+bias on the Scalar engine.  Self-loops are ordinary edges.
- Window block counts vary per window (max over cores keeps the SPMD
  program identical); gather calls pack GCH blocks flat across windows.
- Host prep is integer-only; rsqrt(deg) is computed on device.
"""

import numpy as np
from ml_dtypes import bfloat16, float8_e4m3

N_REAL = 50000
E_REAL = 800000
D = 128
NCORES = 8
GCH = 8  # blocks per dma_gather call (1024 idxs: swdge firmware limit)
PHASES = {"A", "G1", "S", "G2", "H"}

_CFG_FULL = dict(n=N_REAL, nsh=6250, win=256, split=32768)


def _derive(cfg):
    nsh = cfg["nsh"]
    nloc = ((nsh + 127) // 128) * 128
    win = cfg["win"]
    nloc = ((nloc + win - 1) // win) * win
    npad = NCORES * nloc
    return nloc, npad, nloc // win, npad // 128


def _calls(nb):
    """Pack nb blocks into dma_gather calls of <= GCH blocks."""
    out = []
    b0 = 0
    while b0 < nb:
        out.append((b0, min(GCH, nb - b0)))
        b0 += GCH
    return out


def prep(edge_index, cfg=_CFG_FULL):
    """Host-side (integer-only) graph preprocessing -> per-core arrays."""
    n, nsh, win, split = cfg["n"], cfg["nsh"], cfg["win"], cfg["split"]
    nloc, npad, nwin, _ = _derive(cfg)
    row = np.asarray(edge_index[0]).astype(np.int64)
    col = np.asarray(edge_index[1]).astype(np.int64)

    deg = np.bincount(col, minlength=n).astype(np.float32) + 1.0  # + self loop
    deg_t = np.ones(npad, np.float32)
    rr = np.arange(n, dtype=np.int64)
    t_of_r = (rr // nsh) * nloc + (rr % nsh)
    deg_t[t_of_r] = deg

    trow = (row // nsh) * nloc + (row % nsh)
    tcol = (col // nsh) * nloc + (col % nsh)
    core_of = col // nsh

    # per-core/per-class edge lists sorted by window
    percls = []  # [core][cls] = (src_rows, col_in_window, window_bounds)
    for c in range(NCORES):
        m = core_of == c
        er = np.concatenate([trow[m], c * nloc + np.arange(nsh, dtype=np.int64)])
        ecl = np.concatenate([tcol[m] - c * nloc, np.arange(nsh, dtype=np.int64)])
        w = ecl // win
        is_hi = er >= split
        cls_list = []
        for cls in (0, 1):
            mm = is_hi == bool(cls)
            erc, eclc, wc = er[mm], ecl[mm], w[mm]
            order = np.argsort(wc, kind="stable")
            erc, eclc, wc = erc[order], eclc[order], wc[order]
            bounds = np.searchsorted(wc, np.arange(nwin + 1))
            cls_list.append((erc, eclc, bounds))
        percls.append(cls_list)

    # per-(class, window) block count = max over cores (keeps SPMD structure)
    nbw = np.zeros((2, nwin), np.int64)
    for cls in (0, 1):
        for w in range(nwin):
            mx = max(percls[c][cls][2][w + 1] - percls[c][cls][2][w]
                     for c in range(NCORES))
            nbw[cls, w] = (mx + 127) // 128
    assert (nbw.sum(axis=0) > 0).all()

    starts = [np.concatenate([[0], np.cumsum(nbw[cls])]) for cls in (0, 1)]
    NB = [int(starts[cls][-1]) for cls in (0, 1)]

    one = np.uint8(0x38)  # 1.0 in float8_e4m3
    wins = np.arange(win, dtype=np.int64)

    cores = []
    for c in range(NCORES):
        arrs = {}
        for cls, sfx in ((0, "lo"), (1, "hi")):
            erc, eclc, bounds = percls[c][cls]
            nb = NB[cls]
            src = np.zeros((nb, 128), np.int64)
            cw = np.full((nb, 128), -1, np.int64)
            for w in range(nwin):
                a, b = bounds[w], bounds[w + 1]
                k = b - a
                base = starts[cls][w] * 128
                src.reshape(-1)[base:base + k] = erc[a:b] - (split if cls else 0)
                cw.reshape(-1)[base:base + k] = eclc[a:b] % win
            # gather idx layout per call: idx e -> part e%16 (replicated x8),
            # col e//16
            gparts = []
            for b0, cs in _calls(nb):
                s = src[b0:b0 + cs].reshape(-1)
                g16 = s.reshape(-1, 16).T  # [16, cs*8]
                gparts.append(np.tile(g16, (8, 1)))
            arrs[f"gidx_{sfx}"] = np.concatenate(gparts, axis=1).astype(np.int16)
            # one-hot blocks in fp8: [128 part=edge, NB*win], block b at
            # cols [b*win,(b+1)*win)
            oh = (cw[:, :, None] == wins).astype(np.uint8) * one
            oh = oh.transpose(1, 0, 2).reshape(128, nb * win)
            arrs[f"oh_{sfx}"] = oh.view(float8_e4m3)
        # deg of this core's own table rows, [128, nloc/128] tiled
        arrs["degloc"] = deg_t[c * nloc:(c + 1) * nloc].reshape(-1, 128).T.copy()
        # deg of this core's targets broadcast across partitions
        arrs["degb"] = np.tile(deg_t[c * nloc:(c + 1) * nloc], (128, 1)).copy()
        cores.append(arrs)

    return cores, nbw, t_of_r


def build_nc(nbw, cfg=_CFG_FULL):
    import concourse.bacc as bacc
    import concourse.tile as tile
    import concourse.mybir as mybir
    from concourse.alu_op_type import AluOpType

    nloc, npad, nwin, ntile = _derive(cfg)
    win, split = cfg["win"], cfg["split"]
    f32 = mybir.dt.float32
    bf16 = mybir.dt.bfloat16
    f8 = mybir.dt.float8e4
    i16 = mybir.dt.int16
    AF = mybir.ActivationFunctionType
    nsh_t = nloc // 128

    starts = [np.concatenate([[0], np.cumsum(nbw[cls])]) for cls in (0, 1)]
    NB = [int(starts[cls][-1]) for cls in (0, 1)]
    calls = [_calls(NB[cls]) for cls in (0, 1)]
    call_of_block = []
    for cls in (0, 1):
        m = {}
        for j, (b0, cs) in enumerate(calls[cls]):
            for s in range(cs):
                m[b0 + s] = (j, s)
        call_of_block.append(m)

    nc = bacc.Bacc("TRN2", target_bir_lowering=False, debug=False,
                   num_devices=NCORES)
    inp = {}

    def I(name, shape, dt=f32):
        inp[name] = nc.dram_tensor(name, list(shape), dt, kind="ExternalInput").ap()
        return inp[name]

    xloc = I("xloc", [128, nloc], bf16)
    W1 = I("W1", [128, 128], bf16); W2 = I("W2", [128, 128], bf16)
    Wh = I("Wh", [128, 3], bf16)
    b1 = I("b1", [128, 1]); b2 = I("b2", [128, 1]); bh = I("bh", [128, 3])
    degloc = I("degloc", [128, nsh_t]); degb = I("degb", [128, nloc])
    g_lo = I("gidx_lo", [128, NB[0] * 8], i16)
    g_hi = I("gidx_hi", [128, NB[1] * 8], i16)
    oh_d = [I("oh_lo", [128, NB[0] * win], f8), I("oh_hi", [128, NB[1] * win], f8)]
    out = nc.dram_tensor("out", [128, nsh_t * 3], f32, kind="ExternalOutput").ap()

    bounce1 = nc.dram_tensor("bounce1", [nloc, 128], bf16, kind="Internal").ap()
    bounce2 = nc.dram_tensor("bounce2", [nloc, 128], bf16, kind="Internal").ap()
    T1 = nc.dram_tensor("T1", [npad, 128], bf16, kind="Internal",
                        addr_space="Shared").ap()
    T2 = nc.dram_tensor("T2", [npad, 128], bf16, kind="Internal",
                        addr_space="Shared").ap()

    with tile.TileContext(nc) as tc:
        with (
            tc.tile_pool(name="const", bufs=1) as pc,
            tc.tile_pool(name="ha", bufs=3) as pha,
            tc.tile_pool(name="glo", bufs=6) as pglo,
            tc.tile_pool(name="ghi", bufs=6) as pghi,
            tc.tile_pool(name="ohl", bufs=6) as pohl,
            tc.tile_pool(name="ohh", bufs=6) as pohh,
            tc.tile_pool(name="tmp", bufs=2) as ptmp,
            tc.tile_pool(name="act", bufs=1) as pact,
            tc.tile_pool(name="psA", bufs=2, space="PSUM") as psA,
            tc.tile_pool(name="psW", bufs=2, space="PSUM") as psW,
            tc.tile_pool(name="psH", bufs=2, space="PSUM") as psH,
        ):
            def load(ap, shape, tag, dt=f32):
                t = pc.tile(shape, dt, tag=tag)
                nc.sync.dma_start(t[:], ap[:])
                return t

            W1_sb = load(W1, [128, 128], "W1", bf16)
            W2_sb = load(W2, [128, 128], "W2", bf16)
            Wh_sb = load(Wh, [128, 3], "Wh", bf16)
            b1_sb = load(b1, [128, 1], "b1"); b2_sb = load(b2, [128, 1], "b2")
            bh_sb = load(bh, [128, 3], "bh")
            glo_sb = load(g_lo, [128, NB[0] * 8], "glosb", i16)
            ghi_sb = load(g_hi, [128, NB[1] * 8], "ghisb", i16)
            xloc_sb = load(xloc, [128, nloc], "xloc", bf16)

            def rsqrt_of(ap, cols, tag):
                dsb = load(ap, [128, cols], tag + "_d")
                nc.vector.reciprocal(dsb[:], dsb[:])
                nc.scalar.activation(dsb[:], dsb[:], AF.Sqrt)
                return dsb

            disloc_sb = rsqrt_of(degloc, nsh_t, "disl")
            disb_sb = rsqrt_of(degb, nloc, "disb")

            # persistent activations (feature-major)
            x2T = pact.tile([128, nloc], bf16, tag="x2T")
            x3T = pact.tile([128, nloc], bf16, tag="x3T")
            out_sb = pact.tile([128, nsh_t * 3], f32, tag="osb")

            # ---- local table shard: rows t*128..t*128+127 of this core ----
            def table_shard(src_sb, W_sb, bounce):
                for t in range(nsh_t):
                    ps = psA.tile([128, 128], f32, tag="psA")
                    nc.tensor.matmul(ps[:], src_sb[:, t * 128:(t + 1) * 128],
                                     W_sb[:], start=True, stop=True)
                    h = pha.tile([128, 128], bf16, tag="ha")
                    nc.vector.tensor_scalar(h[:], ps[:], disloc_sb[:, t:t + 1],
                                            None, AluOpType.mult)
                    nc.sync.dma_start(bounce[t * 128:(t + 1) * 128, :], h[:])

            def allgather(bounce, T):
                nc.gpsimd.collective_compute(
                    "AllGather", mybir.AluOpType.bypass,
                    replica_groups=[list(range(NCORES))],
                    ins=[bounce[:]], outs=[T[:]])

            if "A" in PHASES:
                table_shard(xloc_sb, W1_sb, bounce1)
                allgather(bounce1, T1)

            # ---- one GCN aggregation layer ----
            def agg_layer(T, xTnext, bias_sb):
                emitted = [{}, {}]  # cls -> call j -> (gather tile, onehot tile)

                def ensure_call(cls, j):
                    if j in emitted[cls]:
                        return emitted[cls][j]
                    b0, cs = calls[cls][j]
                    pl, pohx = (pglo, pohl) if cls == 0 else (pghi, pohh)
                    gsb = glo_sb if cls == 0 else ghi_sb
                    lim = (0, split) if cls == 0 else (split, npad)
                    gt = pl.tile([128, cs, 128], bf16, tag=f"g{cls}")
                    nc.gpsimd.dma_gather(
                        gt[:], T[lim[0]:lim[1], :], gsb[:, b0 * 8:(b0 + cs) * 8],
                        num_idxs=cs * 128, num_idxs_reg=cs * 128,
                        elem_size=128)
                    ot = pohx.tile([128, cs * win], f8, tag=f"o{cls}")
                    nc.sync.dma_start(ot[:], oh_d[cls][:, b0 * win:(b0 + cs) * win])
                    emitted[cls][j] = (gt, ot)
                    return gt, ot

                for w in range(nwin):
                    parts = []
                    for cls in (0, 1):
                        for b in range(int(starts[cls][w]), int(starts[cls][w + 1])):
                            j, slot = call_of_block[cls][b]
                            gt, ot = ensure_call(cls, j)
                            parts.append((gt, ot, slot))
                    acc = psW.tile([128, win], f32, tag="acc")
                    for k, (gt, ot, slot) in enumerate(parts):
                        nc.tensor.matmul(acc[:], gt[:, slot, :],
                                         ot[:, slot * win:(slot + 1) * win],
                                         start=(k == 0), stop=(k == len(parts) - 1))
                    tmp = ptmp.tile([128, win], bf16, tag="tmp")
                    nc.vector.tensor_tensor(tmp[:], acc[:],
                                            disb_sb[:, w * win:(w + 1) * win],
                                            AluOpType.mult)
                    nc.scalar.activation(xTnext[:, w * win:(w + 1) * win],
                                         tmp[:], AF.Relu, bias=bias_sb[:, 0:1])

            if "G1" in PHASES:
                agg_layer(T1, x2T, b1_sb)
            if "S" in PHASES:
                table_shard(x2T, W2_sb, bounce2)
                allgather(bounce2, T2)
            if "G2" in PHASES:
                agg_layer(T2, x3T, b2_sb)

            # ---- head ----
            for t in range(nsh_t if "H" in PHASES else 0):
                ps = psH.tile([128, 3], f32, tag="psH")
                nc.tensor.matmul(ps[:], x3T[:, t * 128:(t + 1) * 128], Wh_sb[:],
                                 start=True, stop=True)
                nc.vector.tensor_tensor(out_sb[:, t * 3:(t + 1) * 3], ps[:],
                                        bh_sb[:], AluOpType.add)
            nc.sync.dma_start(out[:], out_sb[:])

    nc.compile()
    return nc


def kernel(x, edge_index, W1, b1, W2, b2, Wh, bh, cfg=_CFG_FULL, _trace=False):
    from concourse.bass_utils import run_bass_kernel_spmd

    x = np.asarray(x, dtype=np.float32)
    W1 = np.asarray(W1, np.float32); b1 = np.asarray(b1, np.float32)
    W2 = np.asarray(W2, np.float32); b2 = np.asarray(b2, np.float32)
    Wh = np.asarray(Wh, np.float32); bh = np.asarray(bh, np.float32)
    n, nsh, win = cfg["n"], cfg["nsh"], cfg["win"]
    nloc, npad, nwin, ntile = _derive(cfg)

    cores, nbw, t_of_r = prep(edge_index, cfg)
    nc = build_nc(nbw, cfg)

    xTp = np.zeros((128, npad), np.float32)
    xTp[:, t_of_r] = x.T  # table-order, feature-major
    shared = dict(
        W1=W1.astype(bfloat16), W2=W2.astype(bfloat16),
        Wh=Wh.astype(bfloat16),
        b1=b1.reshape(128, 1), b2=b2.reshape(128, 1),
        bh=np.tile(bh.reshape(1, 3), (128, 1)).copy(),
    )
    in_maps = []
    for c in range(NCORES):
        m = dict(shared, **cores[c])
        m["xloc"] = xTp[:, c * nloc:(c + 1) * nloc].astype(bfloat16)
        in_maps.append(m)
    res = run_bass_kernel_spmd(nc, in_maps, core_ids=list(range(NCORES)),
                               trace=_trace)

    outs = []
    for c in range(NCORES):
        o = res.results[c]["out"].reshape(128, nloc // 128, 3)
        outs.append(o.transpose(1, 0, 2).reshape(nloc, 3)[:nsh])
    full = np.concatenate(outs, axis=0)[:n]
    if _trace:
        kernel.last_exec_ns = res.exec_time_ns
        kernel.last_trace = (res.instructions_and_trace or (None, None))[1]
    return full
